# revision 24
# baseline (speedup 1.0000x reference)
"""Trainium2 Bass kernel for sparse per-edge dot-product attention
(GNN message passing) on 8 NeuronCores.

Strategy v2 (gather-free, host-pregathered edge streams):
  - score_e,h = q[row_e]_h . k[col_e]_h = xr_e^T A_h xc_e + v_h^T xc_e
    + (terms constant within a row segment, dropped: softmax-invariant),
    with A_h = Wq_h Wk_h^T [64x64], v_h = Wk_h bq_h.
  - The host sorts edges by source row (padded into 64 stretches x 2048
    slots, no row-run split across a stretch), then ships PRE-GATHERED
    feature streams XR' = [x[row]; 1] (65 x EPAD) and XC = x[col]
    (64 x EPAD) in fp16.  No on-device gathers at all (the v1 per-edge
    dma_gather was GPSIMD-descriptor-generation bound at ~8 ns/edge).
  - Device per stretch: T = Acat^T @ XR' on the PE (Acat [65,128] holds
    both heads; stationary weights), P = T * XC elementwise on DVE,
    per-head feature reduction via a small ones-mask matmul, scores to
    DRAM.
  - Phase 3 (unchanged from v1): per-row softmax denominators via
    hardware segmented scans (forward + reversed broadcast), normalize,
    average heads.  Host scatters padded per-edge results back.
"""

import numpy as np
import ml_dtypes

N = 100000
F = 64
H = 2
E = 1000000
NCORES = 8
NLOC = N // NCORES            # 12500 source nodes per core
ST = 64                       # stretches per core
SLOT = 2048                   # edge slots per stretch
EPAD = ST * SLOT              # 131072 padded edge slots per core
CH = 4                        # 512-col chunks per stretch
BLK = 4                       # stretches per input DMA block

_compiled = {}


def _build_program():
    import concourse.bacc as bacc
    import concourse.mybir as mybir
    import concourse.bass as bass
    from concourse.tile import TileContext

    f32 = mybir.dt.float32
    f16 = mybir.dt.float16
    bf16 = mybir.dt.bfloat16

    nc = bacc.Bacc()

    # ---- inputs ----
    xr = nc.dram_tensor("xr", [F + 1, EPAD], f16, kind="ExternalInput")
    xc = nc.dram_tensor("xc", [F, EPAD], f16, kind="ExternalInput")
    acat = nc.dram_tensor("acat", [F + 1, 2 * F], f16, kind="ExternalInput")
    ones4 = nc.dram_tensor("ones4", [128, CH, 2 * CH], f16, kind="ExternalInput")
    segm = nc.dram_tensor("segm", [ST, SLOT], bf16, kind="ExternalInput")

    # ---- internal DRAM ----
    NG = ST // BLK   # 4-stretch groups
    sdram = nc.dram_tensor("sdram", [NG, 2, 64, 512], f32, kind="Internal")

    # ---- output ----
    attn_out = nc.dram_tensor("attn", [ST, SLOT], f32, kind="ExternalOutput")

    AP = bass.AP

    # ============ Phase 2: scores = xr^T A xc (streamed) ============
    with TileContext(nc) as tc:
        with (
            tc.tile_pool(name="cst", bufs=1) as cpool,
            tc.tile_pool(name="xin", bufs=2) as xpool,
            tc.tile_pool(name="wrk", bufs=2) as wpool,
            tc.tile_pool(name="tps", bufs=2, space="PSUM") as tpool,
            tc.tile_pool(name="sps", bufs=2, space="PSUM") as spool,
        ):
            a_t = cpool.tile([F + 1, 2 * F], f16)
            o_t = cpool.tile([128, CH, 2 * CH], f16)
            nc.sync.dma_start(out=a_t[:], in_=acat[:])
            nc.sync.dma_start(out=o_t[:], in_=ones4[:])

            HS = SLOT // 2   # 1024-col half stretches
            for b0 in range(0, ST, BLK):
                xr_t = xpool.tile([F + 1, BLK * SLOT], f16, tag="xr")
                xc_t = xpool.tile([128, BLK * SLOT], f16, tag="xc")
                nc.sync.dma_start(out=xr_t[:], in_=xr[:, b0 * SLOT:(b0 + BLK) * SLOT])
                nc.sync.dma_start(out=xc_t[:F, :], in_=xc[:, b0 * SLOT:(b0 + BLK) * SLOT])
                # duplicate col features into the upper head half (GPSIMD)
                nc.gpsimd.tensor_copy(out=xc_t[F:, :], in_=xc_t[:F, :])
                # one score psum tile per stretch pair: stretch k2 of the
                # pair writes rows [32*k2, 32*k2+8) (psum base must be 0/32)
                sc_ps = None
                for si in range(BLK):
                    if si % 2 == 0:
                        sc_ps = spool.tile([64, 512], f32, tag="sc")
                    p_t = wpool.tile([128, SLOT], f16, tag="p")
                    # T[64h+f, e] = sum_g Acat[g, 64h+f] * xr[g, e]
                    # (double-buffered [128, 1024] psum tiles; one 512-col
                    #  matmul per bank half, P-mult at 1024 granularity)
                    for j in range(2):
                        e0 = si * SLOT + j * HS
                        t_ps = tpool.tile([128, HS], f32, tag="tps")
                        for u in range(2):
                            nc.tensor.matmul(
                                t_ps[:, 512 * u:512 * (u + 1)],
                                lhsT=a_t[:],
                                rhs=xr_t[:, e0 + 512 * u:e0 + 512 * u + 512],
                                start=True, stop=True,
                            )
                        nc.vector.tensor_tensor(
                            out=p_t[:, j * HS:(j + 1) * HS],
                            in0=t_ps[:],
                            in1=xc_t[:, e0:e0 + HS],
                            op=mybir.AluOpType.mult)
                    # scores: row 32*(si%2)+2c+h = sum_f P[64h+f, 512c+i]
                    r0 = 32 * (si % 2)
                    for c in range(CH):
                        nc.tensor.matmul(
                            sc_ps[r0:r0 + 8, :],
                            lhsT=o_t[:, c, :],
                            rhs=p_t[:, 512 * c:512 * (c + 1)],
                            start=(c == 0), stop=(c == CH - 1),
                        )
                    if si % 2 == 1:
                        sc_t = wpool.tile([64, 512], f32, tag="sct")
                        nc.scalar.activation(out=sc_t[:], in_=sc_ps[:],
                                             func=mybir.ActivationFunctionType.Copy)
                        nc.sync.dma_start(out=sdram[b0 // BLK, si // 2, :, :],
                                          in_=sc_t[:])

    # ============ Phase 3: segmented softmax (unchanged from v1) ========
    with TileContext(nc) as tc:
        with tc.tile_pool(name="p3", bufs=1) as pool:
            s_all = pool.tile([128, SLOT], f32)
            sm = pool.tile([128, SLOT], bf16)
            # partition q = 64h + 16k + g holds stretch st = 4g + k, head h:
            # with k = 2*half + k2: s_all[q, 512c+i] = sdram[g, half, 32k2+2c+h, i]
            for h in range(2):
                for k in range(BLK):
                    half, k2 = divmod(k, 2)
                    q0 = 64 * h + 16 * k
                    nc.sync.dma_start(
                        out=s_all[q0:q0 + 16, :].rearrange("p (a b) -> p a b", a=CH),
                        in_=AP(sdram, half * 32768 + (32 * k2 + h) * 512,
                               [[2 * 64 * 512, 16], [2 * 512, CH], [1, 512]]))
                    nc.sync.dma_start(
                        out=sm[q0:q0 + 16, :],
                        in_=AP(segm, k * SLOT, [[BLK * SLOT, 16], [1, SLOT]]))

            ex = pool.tile([128, SLOT], f32)
            nc.scalar.activation(out=ex[:], in_=s_all[:],
                                 func=mybir.ActivationFunctionType.Exp)
            # forward segmented scan: state = m*state + e
            scf = pool.tile([128, SLOT], f32)
            nc.vector.tensor_tensor_scan(scf[:], sm[:], ex[:], 0.0,
                                         mybir.AluOpType.mult, mybir.AluOpType.add)
            # m_next (shift left by 1, last=0) and (1-m_next)*scf
            mnx = pool.tile([128, SLOT], f32)
            nc.vector.memset(mnx[:, SLOT - 1:SLOT], 0)
            nc.vector.tensor_copy(out=mnx[:, :SLOT - 1], in_=sm[:, 1:])
            omn = pool.tile([128, SLOT], f32)
            nc.vector.tensor_scalar(out=omn[:], in0=mnx[:], scalar1=-1.0, scalar2=1.0,
                                    op0=mybir.AluOpType.mult, op1=mybir.AluOpType.add)
            d1b = pool.tile([128, SLOT], f32)
            nc.vector.tensor_tensor(out=d1b[:], in0=omn[:], in1=scf[:],
                                    op=mybir.AluOpType.mult)
            # backward scan (reversed APs): state = mnx*state + d1b
            den = pool.tile([128, SLOT], f32)

            def rev(ap):
                (ps_, pc_), (fs_, fc_) = ap.ap
                return AP(ap.tensor, ap.offset + fs_ * (fc_ - 1),
                          [[ps_, pc_], [-fs_, fc_]])

            nc.vector.tensor_tensor_scan(rev(den[:]), rev(mnx[:]), rev(d1b[:]), 0.0,
                                         mybir.AluOpType.mult, mybir.AluOpType.add)
            rd = pool.tile([128, SLOT], f32)
            nc.vector.reciprocal(out=rd[:], in_=den[:])
            at = pool.tile([128, SLOT], f32)
            nc.vector.tensor_tensor(out=at[:], in0=ex[:], in1=rd[:],
                                    op=mybir.AluOpType.mult)
            h1 = pool.tile([64, SLOT], f32)
            nc.vector.tensor_copy(out=h1[:], in_=at[64:128, :])
            mn = pool.tile([64, SLOT], f32)
            nc.vector.tensor_tensor(out=mn[:], in0=at[0:64, :], in1=h1[:],
                                    op=mybir.AluOpType.add)
            nc.vector.tensor_scalar_mul(mn[:], mn[:], 0.5)
            # mn partition 16k+g -> stretch st = 4g+k
            for k in range(BLK):
                nc.sync.dma_start(
                    out=AP(attn_out, k * SLOT, [[BLK * SLOT, 16], [1, SLOT]]),
                    in_=mn[16 * k:16 * k + 16, :])

    nc.finalize()
    return nc


def _prep_core(row, col, eid, n0):
    """Sort by row and pack runs into ST stretches of SLOT slots without
    splitting a run; returns padded slot_row/slot_col/slot_eid + segm."""
    order = np.argsort(row, kind="stable")
    row = row[order]
    col = col[order]
    eid = eid[order]

    counts = np.bincount(row - n0, minlength=NLOC)
    slot_row = np.full(EPAD, -1, np.int64)
    slot_col = np.zeros(EPAD, np.int64)
    slot_eid = np.full(EPAD, -1, np.int64)
    pos = 0
    src = 0
    for node in range(NLOC):
        d = counts[node]
        if d == 0:
            continue
        if (pos % SLOT) + d > SLOT:
            pos = ((pos // SLOT) + 1) * SLOT
        assert pos + d <= EPAD, "edge padding overflow"
        slot_row[pos:pos + d] = row[src:src + d]
        slot_col[pos:pos + d] = col[src:src + d]
        slot_eid[pos:pos + d] = eid[src:src + d]
        pos += d
        src += d

    r2 = slot_row.reshape(ST, SLOT)
    segm = np.zeros((ST, SLOT), ml_dtypes.bfloat16)
    same = (r2[:, 1:] == r2[:, :-1]) & (r2[:, 1:] >= 0)
    segm[:, 1:] = same.astype(ml_dtypes.bfloat16)
    return slot_row, slot_col, slot_eid, segm


def kernel(x, W, b, edge_index):
    from concourse.bass_utils import run_bass_kernel_spmd

    x = np.asarray(x, np.float32)
    W = np.asarray(W, np.float32)
    b = np.asarray(b, np.float32)
    edge_index = np.asarray(edge_index, np.int32)

    if "nc" not in _compiled:
        _compiled["nc"] = _build_program()
    nc = _compiled["nc"]

    # fused score matrices: per head h, Acat[:, 64h:64h+64] = [A_h; v_h^T]
    acat = np.zeros((F + 1, 2 * F), np.float64)
    for h in range(H):
        Wq = W[:, 128 * h:128 * h + 64].astype(np.float64)
        bq = b[128 * h:128 * h + 64].astype(np.float64)
        Wk = W[:, 128 * h + 64:128 * h + 128].astype(np.float64)
        acat[:F, 64 * h:64 * h + 64] = Wq @ Wk.T
        acat[F, 64 * h:64 * h + 64] = Wk @ bq
    acat = acat.astype(np.float16)

    # ones reduction masks: lhsT for chunk c maps head h -> out row 2c+h
    ones4 = np.zeros((128, CH, 2 * CH), np.float16)
    p = np.arange(128)
    for c in range(CH):
        ones4[p, c, 2 * c + (p // 64)] = 1.0

    row = edge_index[0].astype(np.int64)
    col = edge_index[1].astype(np.int64)
    core_of = row // NLOC
    eids = np.arange(E, dtype=np.int64)
    xT = np.ascontiguousarray(x.T)  # [F, N] f32

    in_maps = []
    slot_eids = []
    for c in range(NCORES):
        msk = core_of == c
        n0 = c * NLOC
        slot_row, slot_col, slot_eid, segm = _prep_core(
            row[msk], col[msk], eids[msk], n0)
        real = slot_row >= 0
        xr_s = np.zeros((F + 1, EPAD), np.float16)
        xc_s = np.zeros((F, EPAD), np.float16)
        xr_s[:F, real] = xT[:, slot_row[real]].astype(np.float16)
        xr_s[F, real] = 1.0
        xc_s[:, real] = xT[:, slot_col[real]].astype(np.float16)
        in_maps.append({
            "xr": xr_s, "xc": xc_s, "acat": acat, "ones4": ones4, "segm": segm,
        })
        slot_eids.append(slot_eid)

    res = run_bass_kernel_spmd(nc, in_maps, core_ids=list(range(NCORES)),
                               trace=bool(_compiled.get("trace")))
    _compiled["last_result"] = res

    out = np.zeros(E, np.float32)
    for c in range(NCORES):
        a = np.asarray(res.results[c]["attn"]).reshape(EPAD)
        se = slot_eids[c]
        m = se >= 0
        out[se[m]] = a[m]
    return out


# revision 25
# speedup vs baseline: 1.3846x; 1.3846x over previous
"""Trainium2 Bass kernel for sparse per-edge dot-product attention
(GNN message passing) on 8 NeuronCores.

Strategy v2 (gather-free, host-pregathered edge streams):
  - score_e,h = q[row_e]_h . k[col_e]_h = xr_e^T A_h xc_e + v_h^T xc_e
    + (terms constant within a row segment, dropped: softmax-invariant),
    with A_h = Wq_h Wk_h^T [64x64], v_h = Wk_h bq_h.
  - The host sorts edges by source row (padded into 64 stretches x 2048
    slots, no row-run split across a stretch), then ships PRE-GATHERED
    feature streams XR' = [x[row]; 1] (65 x EPAD) and XC = x[col]
    (64 x EPAD) in fp16.  No on-device gathers at all (the v1 per-edge
    dma_gather was GPSIMD-descriptor-generation bound at ~8 ns/edge).
  - Device per stretch: T = Acat^T @ XR' on the PE (Acat [65,128] holds
    both heads; stationary weights), P = T * XC elementwise on DVE,
    per-head feature reduction via a small ones-mask matmul, scores to
    DRAM.
  - Phase 3 (unchanged from v1): per-row softmax denominators via
    hardware segmented scans (forward + reversed broadcast), normalize,
    average heads.  Host scatters padded per-edge results back.
"""

import numpy as np
import ml_dtypes

N = 100000
F = 64
H = 2
E = 1000000
NCORES = 8
NLOC = N // NCORES            # 12500 source nodes per core
ST = 64                       # stretches per core
SLOT = 2048                   # edge slots per stretch
EPAD = ST * SLOT              # 131072 padded edge slots per core
CH = 4                        # 512-col chunks per stretch
BLK = 4                       # stretches per input DMA block

_compiled = {}


def _build_program():
    import concourse.bacc as bacc
    import concourse.mybir as mybir
    import concourse.bass as bass
    from concourse.tile import TileContext

    f32 = mybir.dt.float32
    f16 = mybir.dt.float16
    bf16 = mybir.dt.bfloat16

    nc = bacc.Bacc()

    # ---- inputs ----
    xr = nc.dram_tensor("xr", [F + 1, EPAD], f16, kind="ExternalInput")
    xc = nc.dram_tensor("xc", [F, EPAD], f16, kind="ExternalInput")
    acat = nc.dram_tensor("acat", [F + 1, 2 * F], f16, kind="ExternalInput")
    ones4 = nc.dram_tensor("ones4", [128, CH, 2 * CH], f16, kind="ExternalInput")
    segm = nc.dram_tensor("segm", [ST, SLOT], bf16, kind="ExternalInput")

    # ---- internal DRAM ----
    sdram = nc.dram_tensor("sdram", [ST, 2 * CH, 512], f32, kind="Internal")

    # ---- output ----
    attn_out = nc.dram_tensor("attn", [ST, SLOT], f32, kind="ExternalOutput")

    AP = bass.AP

    # ============ Phase 2: scores = xr^T A xc (streamed) ============
    with TileContext(nc) as tc:
        with (
            tc.tile_pool(name="cst", bufs=1) as cpool,
            tc.tile_pool(name="xin", bufs=2) as xpool,
            tc.tile_pool(name="wrk", bufs=2) as wpool,
            tc.tile_pool(name="tps", bufs=2, space="PSUM") as tpool,
            tc.tile_pool(name="sps", bufs=2, space="PSUM") as spool,
        ):
            a_t = cpool.tile([F + 1, 2 * F], f16)
            o_t = cpool.tile([128, CH, 2 * CH], f16)
            nc.sync.dma_start(out=a_t[:], in_=acat[:])
            nc.sync.dma_start(out=o_t[:], in_=ones4[:])

            HS = SLOT // 2   # 1024-col half stretches
            for b0 in range(0, ST, BLK):
                xr_t = xpool.tile([F + 1, BLK * SLOT], f16, tag="xr")
                xc_t = xpool.tile([128, BLK * SLOT], f16, tag="xc")
                nc.sync.dma_start(out=xr_t[:], in_=xr[:, b0 * SLOT:(b0 + BLK) * SLOT])
                nc.sync.dma_start(out=xc_t[:F, :], in_=xc[:, b0 * SLOT:(b0 + BLK) * SLOT])
                # duplicate col features into the upper head half (ACT engine)
                nc.scalar.activation(out=xc_t[F:, :], in_=xc_t[:F, :],
                                     func=mybir.ActivationFunctionType.Copy)
                for si in range(BLK):
                    p_t = wpool.tile([128, SLOT], f16, tag="p")
                    # T[64h+f, e] = sum_g Acat[g, 64h+f] * xr[g, e]
                    # (double-buffered [128, 1024] psum tiles; one 512-col
                    #  matmul per bank half, P-mult at 1024 granularity)
                    for j in range(2):
                        e0 = si * SLOT + j * HS
                        t_ps = tpool.tile([128, HS], f32, tag="tps")
                        for u in range(2):
                            nc.tensor.matmul(
                                t_ps[:, 512 * u:512 * (u + 1)],
                                lhsT=a_t[:],
                                rhs=xr_t[:, e0 + 512 * u:e0 + 512 * u + 512],
                                start=True, stop=True,
                            )
                        nc.vector.tensor_tensor(
                            out=p_t[:, j * HS:(j + 1) * HS],
                            in0=t_ps[:],
                            in1=xc_t[:, e0:e0 + HS],
                            op=mybir.AluOpType.mult)
                    # scores: row 2c+h of [8, 512] = sum_f P[64h+f, 512c+i]
                    sc_ps = spool.tile([2 * CH, 512], f32, tag="sc")
                    for c in range(CH):
                        nc.tensor.matmul(
                            sc_ps[:],
                            lhsT=o_t[:, c, :],
                            rhs=p_t[:, 512 * c:512 * (c + 1)],
                            start=(c == 0), stop=(c == CH - 1),
                        )
                    sc_t = wpool.tile([2 * CH, 512], f32, tag="sct")
                    nc.scalar.activation(out=sc_t[:], in_=sc_ps[:],
                                         func=mybir.ActivationFunctionType.Copy)
                    nc.sync.dma_start(out=sdram[b0 + si, :, :], in_=sc_t[:])

    # ============ Phase 3: segmented softmax (unchanged from v1) ========
    with TileContext(nc) as tc:
        with tc.tile_pool(name="p3", bufs=1) as pool:
            s_all = pool.tile([128, SLOT], f32)
            sm = pool.tile([128, SLOT], bf16)
            # s_all[64h+st, 512c+i] = sdram[st, 2c+h, i]
            for h in range(2):
                nc.sync.dma_start(
                    out=s_all[64 * h:64 * h + 64, :].rearrange("p (a b) -> p a b", a=CH),
                    in_=AP(sdram, h * 512,
                           [[2 * CH * 512, ST], [2 * 512, CH], [1, 512]]))
                nc.sync.dma_start(out=sm[64 * h:64 * h + 64, :], in_=segm[:])

            ex = pool.tile([128, SLOT], f32)
            nc.scalar.activation(out=ex[:], in_=s_all[:],
                                 func=mybir.ActivationFunctionType.Exp)
            # forward segmented scan: state = m*state + e
            scf = pool.tile([128, SLOT], f32)
            nc.vector.tensor_tensor_scan(scf[:], sm[:], ex[:], 0.0,
                                         mybir.AluOpType.mult, mybir.AluOpType.add)
            # m_next (shift left by 1, last=0) and (1-m_next)*scf
            mnx = pool.tile([128, SLOT], f32)
            nc.vector.memset(mnx[:, SLOT - 1:SLOT], 0)
            nc.vector.tensor_copy(out=mnx[:, :SLOT - 1], in_=sm[:, 1:])
            omn = pool.tile([128, SLOT], f32)
            nc.vector.tensor_scalar(out=omn[:], in0=mnx[:], scalar1=-1.0, scalar2=1.0,
                                    op0=mybir.AluOpType.mult, op1=mybir.AluOpType.add)
            d1b = pool.tile([128, SLOT], f32)
            nc.vector.tensor_tensor(out=d1b[:], in0=omn[:], in1=scf[:],
                                    op=mybir.AluOpType.mult)
            # backward scan (reversed APs): state = mnx*state + d1b
            den = pool.tile([128, SLOT], f32)

            def rev(ap):
                (ps_, pc_), (fs_, fc_) = ap.ap
                return AP(ap.tensor, ap.offset + fs_ * (fc_ - 1),
                          [[ps_, pc_], [-fs_, fc_]])

            nc.vector.tensor_tensor_scan(rev(den[:]), rev(mnx[:]), rev(d1b[:]), 0.0,
                                         mybir.AluOpType.mult, mybir.AluOpType.add)
            rd = pool.tile([128, SLOT], f32)
            nc.vector.reciprocal(out=rd[:], in_=den[:])
            at = pool.tile([128, SLOT], f32)
            nc.vector.tensor_tensor(out=at[:], in0=ex[:], in1=rd[:],
                                    op=mybir.AluOpType.mult)
            h1 = pool.tile([64, SLOT], f32)
            nc.vector.tensor_copy(out=h1[:], in_=at[64:128, :])
            mn = pool.tile([64, SLOT], f32)
            nc.vector.tensor_tensor(out=mn[:], in0=at[0:64, :], in1=h1[:],
                                    op=mybir.AluOpType.add)
            nc.vector.tensor_scalar_mul(mn[:], mn[:], 0.5)
            nc.sync.dma_start(out=attn_out[:], in_=mn[:])

    nc.finalize()
    return nc


def _prep_core(row, col, eid, n0):
    """Sort by row and pack runs into ST stretches of SLOT slots without
    splitting a run; returns padded slot_row/slot_col/slot_eid + segm."""
    order = np.argsort(row, kind="stable")
    row = row[order]
    col = col[order]
    eid = eid[order]

    counts = np.bincount(row - n0, minlength=NLOC)
    slot_row = np.full(EPAD, -1, np.int64)
    slot_col = np.zeros(EPAD, np.int64)
    slot_eid = np.full(EPAD, -1, np.int64)
    pos = 0
    src = 0
    for node in range(NLOC):
        d = counts[node]
        if d == 0:
            continue
        if (pos % SLOT) + d > SLOT:
            pos = ((pos // SLOT) + 1) * SLOT
        assert pos + d <= EPAD, "edge padding overflow"
        slot_row[pos:pos + d] = row[src:src + d]
        slot_col[pos:pos + d] = col[src:src + d]
        slot_eid[pos:pos + d] = eid[src:src + d]
        pos += d
        src += d

    r2 = slot_row.reshape(ST, SLOT)
    segm = np.zeros((ST, SLOT), ml_dtypes.bfloat16)
    same = (r2[:, 1:] == r2[:, :-1]) & (r2[:, 1:] >= 0)
    segm[:, 1:] = same.astype(ml_dtypes.bfloat16)
    return slot_row, slot_col, slot_eid, segm


def kernel(x, W, b, edge_index):
    from concourse.bass_utils import run_bass_kernel_spmd

    x = np.asarray(x, np.float32)
    W = np.asarray(W, np.float32)
    b = np.asarray(b, np.float32)
    edge_index = np.asarray(edge_index, np.int32)

    if "nc" not in _compiled:
        _compiled["nc"] = _build_program()
    nc = _compiled["nc"]

    # fused score matrices: per head h, Acat[:, 64h:64h+64] = [A_h; v_h^T]
    acat = np.zeros((F + 1, 2 * F), np.float64)
    for h in range(H):
        Wq = W[:, 128 * h:128 * h + 64].astype(np.float64)
        bq = b[128 * h:128 * h + 64].astype(np.float64)
        Wk = W[:, 128 * h + 64:128 * h + 128].astype(np.float64)
        acat[:F, 64 * h:64 * h + 64] = Wq @ Wk.T
        acat[F, 64 * h:64 * h + 64] = Wk @ bq
    acat = acat.astype(np.float16)

    # ones reduction masks: lhsT for chunk c maps head h -> out row 2c+h
    ones4 = np.zeros((128, CH, 2 * CH), np.float16)
    p = np.arange(128)
    for c in range(CH):
        ones4[p, c, 2 * c + (p // 64)] = 1.0

    row = edge_index[0].astype(np.int64)
    col = edge_index[1].astype(np.int64)
    core_of = row // NLOC
    eids = np.arange(E, dtype=np.int64)
    xT = np.ascontiguousarray(x.T)  # [F, N] f32

    in_maps = []
    slot_eids = []
    for c in range(NCORES):
        msk = core_of == c
        n0 = c * NLOC
        slot_row, slot_col, slot_eid, segm = _prep_core(
            row[msk], col[msk], eids[msk], n0)
        real = slot_row >= 0
        xr_s = np.zeros((F + 1, EPAD), np.float16)
        xc_s = np.zeros((F, EPAD), np.float16)
        xr_s[:F, real] = xT[:, slot_row[real]].astype(np.float16)
        xr_s[F, real] = 1.0
        xc_s[:, real] = xT[:, slot_col[real]].astype(np.float16)
        in_maps.append({
            "xr": xr_s, "xc": xc_s, "acat": acat, "ones4": ones4, "segm": segm,
        })
        slot_eids.append(slot_eid)

    res = run_bass_kernel_spmd(nc, in_maps, core_ids=list(range(NCORES)),
                               trace=bool(_compiled.get("trace")))
    _compiled["last_result"] = res

    out = np.zeros(E, np.float32)
    for c in range(NCORES):
        a = np.asarray(res.results[c]["attn"]).reshape(EPAD)
        se = slot_eids[c]
        m = se >= 0
        out[se[m]] = a[m]
    return out


# revision 29
# speedup vs baseline: 1.9589x; 1.4147x over previous
"""Trainium2 Bass kernel for sparse per-edge dot-product attention
(GNN message passing) on 8 NeuronCores.

Strategy v2 (gather-free, host-pregathered edge streams):
  - score_e,h = q[row_e]_h . k[col_e]_h = xr_e^T A_h xc_e + v_h^T xc_e
    + (terms constant within a row segment, dropped: softmax-invariant),
    with A_h = Wq_h Wk_h^T [64x64], v_h = Wk_h bq_h.
  - The host sorts edges by source row (padded into 64 stretches x 2048
    slots, no row-run split across a stretch), then ships PRE-GATHERED
    feature streams XR' = [x[row]; 1] (65 x EPAD) and XC = x[col]
    (64 x EPAD) in fp16.  No on-device gathers at all (the v1 per-edge
    dma_gather was GPSIMD-descriptor-generation bound at ~8 ns/edge).
  - Device per stretch: T = Acat^T @ XR' on the PE (Acat [65,128] holds
    both heads; stationary weights), P = T * XC elementwise on DVE,
    per-head feature reduction via a small ones-mask matmul, scores to
    DRAM.
  - Phase 3 (unchanged from v1): per-row softmax denominators via
    hardware segmented scans (forward + reversed broadcast), normalize,
    average heads.  Host scatters padded per-edge results back.
"""

import numpy as np
import ml_dtypes

N = 100000
F = 64
H = 2
E = 1000000
NCORES = 8
NLOC = N // NCORES            # 12500 source nodes per core
ST = 64                       # stretches per core
SLOT = 2048                   # edge slots per stretch
EPAD = ST * SLOT              # 131072 padded edge slots per core
CH = 4                        # 512-col chunks per stretch
BLK = 4                       # stretches per input DMA block

_compiled = {}


def _build_program():
    import concourse.bacc as bacc
    import concourse.mybir as mybir
    import concourse.bass as bass
    from concourse.tile import TileContext

    f32 = mybir.dt.float32
    f16 = mybir.dt.float16
    bf16 = mybir.dt.bfloat16

    nc = bacc.Bacc()

    # ---- inputs ----
    xr = nc.dram_tensor("xr", [F + 1, EPAD], f16, kind="ExternalInput")
    xc = nc.dram_tensor("xc", [2 * F, EPAD], f16, kind="ExternalInput")
    acat = nc.dram_tensor("acat", [F + 1, 2 * F], f16, kind="ExternalInput")
    ones4 = nc.dram_tensor("ones4", [128, CH, 2 * CH], f16, kind="ExternalInput")
    segm = nc.dram_tensor("segm", [ST, SLOT], bf16, kind="ExternalInput")

    # ---- internal DRAM ----
    sdram = nc.dram_tensor("sdram", [ST, 2 * CH, 512], f32, kind="Internal")

    # ---- output ----
    attn_out = nc.dram_tensor("attn", [ST, SLOT], f32, kind="ExternalOutput")

    AP = bass.AP

    # ============ Phase 2: scores = xr^T A xc (streamed) ============
    with TileContext(nc) as tc:
        with (
            tc.tile_pool(name="cst", bufs=1) as cpool,
            tc.tile_pool(name="xin", bufs=2) as xpool,
            tc.tile_pool(name="wrk", bufs=2) as wpool,
            tc.tile_pool(name="tps", bufs=3, space="PSUM") as tpool,
            tc.tile_pool(name="sps", bufs=2, space="PSUM") as spool,
        ):
            a_t = cpool.tile([F + 1, 2 * F], f16)
            o_t = cpool.tile([128, CH, 2 * CH], f16)
            nc.sync.dma_start(out=a_t[:], in_=acat[:])
            nc.sync.dma_start(out=o_t[:], in_=ones4[:])

            HS = SLOT // 2   # 1024-col half stretches
            for b0 in range(0, ST, BLK):
                xr_t = xpool.tile([F + 1, BLK * SLOT], f16, tag="xr")
                xc_t = xpool.tile([128, BLK * SLOT], f16, tag="xc")
                nc.sync.dma_start(out=xr_t[:], in_=xr[:, b0 * SLOT:(b0 + BLK) * SLOT])
                nc.sync.dma_start(out=xc_t[:], in_=xc[:, b0 * SLOT:(b0 + BLK) * SLOT])
                for si in range(BLK):
                    p_t = wpool.tile([128, SLOT], f16, tag="p")
                    # T[64h+f, e] = sum_g Acat[g, 64h+f] * xr[g, e]
                    # (double-buffered [128, 1024] psum tiles; one 512-col
                    #  matmul per bank half, P-mult at 1024 granularity)
                    for j in range(2):
                        e0 = si * SLOT + j * HS
                        t_ps = tpool.tile([128, HS], f32, tag="tps")
                        for u in range(2):
                            nc.tensor.matmul(
                                t_ps[:, 512 * u:512 * (u + 1)],
                                lhsT=a_t[:],
                                rhs=xr_t[:, e0 + 512 * u:e0 + 512 * u + 512],
                                start=True, stop=True,
                            )
                        nc.vector.tensor_tensor(
                            out=p_t[:, j * HS:(j + 1) * HS],
                            in0=t_ps[:],
                            in1=xc_t[:, e0:e0 + HS],
                            op=mybir.AluOpType.mult)
                    # scores: row 2c+h of [8, 512] = sum_f P[64h+f, 512c+i]
                    sc_ps = spool.tile([2 * CH, 512], f32, tag="sc")
                    for c in range(CH):
                        nc.tensor.matmul(
                            sc_ps[:],
                            lhsT=o_t[:, c, :],
                            rhs=p_t[:, 512 * c:512 * (c + 1)],
                            start=(c == 0), stop=(c == CH - 1),
                        )
                    sc_t = wpool.tile([2 * CH, 512], f32, tag="sct")
                    nc.scalar.activation(out=sc_t[:], in_=sc_ps[:],
                                         func=mybir.ActivationFunctionType.Copy)
                    nc.sync.dma_start(out=sdram[b0 + si, :, :], in_=sc_t[:])

    # ============ Phase 3: segmented softmax (unchanged from v1) ========
    with TileContext(nc) as tc:
        with tc.tile_pool(name="p3", bufs=1) as pool:
            s_all = pool.tile([128, SLOT], f32)
            sm = pool.tile([128, SLOT], bf16)
            # s_all[64h+st, 512c+i] = sdram[st, 2c+h, i]
            for h in range(2):
                nc.sync.dma_start(
                    out=s_all[64 * h:64 * h + 64, :].rearrange("p (a b) -> p a b", a=CH),
                    in_=AP(sdram, h * 512,
                           [[2 * CH * 512, ST], [2 * 512, CH], [1, 512]]))
                nc.sync.dma_start(out=sm[64 * h:64 * h + 64, :], in_=segm[:])

            ex = pool.tile([128, SLOT], f32)
            nc.scalar.activation(out=ex[:], in_=s_all[:],
                                 func=mybir.ActivationFunctionType.Exp)
            # forward segmented scan: state = m*state + e
            scf = pool.tile([128, SLOT], f32)
            nc.vector.tensor_tensor_scan(scf[:], sm[:], ex[:], 0.0,
                                         mybir.AluOpType.mult, mybir.AluOpType.add)
            # m_next (shift left by 1, last=0) and (1-m_next)*scf
            mnx = pool.tile([128, SLOT], f32)
            nc.vector.memset(mnx[:, SLOT - 1:SLOT], 0)
            nc.vector.tensor_copy(out=mnx[:, :SLOT - 1], in_=sm[:, 1:])
            omn = pool.tile([128, SLOT], f32)
            nc.vector.tensor_scalar(out=omn[:], in0=mnx[:], scalar1=-1.0, scalar2=1.0,
                                    op0=mybir.AluOpType.mult, op1=mybir.AluOpType.add)
            d1b = pool.tile([128, SLOT], f32)
            nc.vector.tensor_tensor(out=d1b[:], in0=omn[:], in1=scf[:],
                                    op=mybir.AluOpType.mult)
            # backward scan (reversed APs): state = mnx*state + d1b
            den = pool.tile([128, SLOT], f32)

            def rev(ap):
                (ps_, pc_), (fs_, fc_) = ap.ap
                return AP(ap.tensor, ap.offset + fs_ * (fc_ - 1),
                          [[ps_, pc_], [-fs_, fc_]])

            nc.vector.tensor_tensor_scan(rev(den[:]), rev(mnx[:]), rev(d1b[:]), 0.0,
                                         mybir.AluOpType.mult, mybir.AluOpType.add)
            rd = pool.tile([128, SLOT], f32)
            nc.vector.reciprocal(out=rd[:], in_=den[:])
            at = pool.tile([128, SLOT], f32)
            nc.vector.tensor_tensor(out=at[:], in0=ex[:], in1=rd[:],
                                    op=mybir.AluOpType.mult)
            h1 = pool.tile([64, SLOT], f32)
            nc.vector.tensor_copy(out=h1[:], in_=at[64:128, :])
            mn = pool.tile([64, SLOT], f32)
            nc.vector.tensor_tensor(out=mn[:], in0=at[0:64, :], in1=h1[:],
                                    op=mybir.AluOpType.add)
            nc.vector.tensor_scalar_mul(mn[:], mn[:], 0.5)
            nc.sync.dma_start(out=attn_out[:], in_=mn[:])

    nc.finalize()
    return nc


def _prep_core(row, col, eid, n0):
    """Sort by row and pack runs into ST stretches of SLOT slots without
    splitting a run; returns padded slot_row/slot_col/slot_eid + segm."""
    order = np.argsort(row, kind="stable")
    row = row[order]
    col = col[order]
    eid = eid[order]

    counts = np.bincount(row - n0, minlength=NLOC)
    slot_row = np.full(EPAD, -1, np.int64)
    slot_col = np.zeros(EPAD, np.int64)
    slot_eid = np.full(EPAD, -1, np.int64)
    pos = 0
    src = 0
    for node in range(NLOC):
        d = counts[node]
        if d == 0:
            continue
        if (pos % SLOT) + d > SLOT:
            pos = ((pos // SLOT) + 1) * SLOT
        assert pos + d <= EPAD, "edge padding overflow"
        slot_row[pos:pos + d] = row[src:src + d]
        slot_col[pos:pos + d] = col[src:src + d]
        slot_eid[pos:pos + d] = eid[src:src + d]
        pos += d
        src += d

    r2 = slot_row.reshape(ST, SLOT)
    segm = np.zeros((ST, SLOT), ml_dtypes.bfloat16)
    same = (r2[:, 1:] == r2[:, :-1]) & (r2[:, 1:] >= 0)
    segm[:, 1:] = same.astype(ml_dtypes.bfloat16)
    return slot_row, slot_col, slot_eid, segm


def kernel(x, W, b, edge_index):
    from concourse.bass_utils import run_bass_kernel_spmd

    x = np.asarray(x, np.float32)
    W = np.asarray(W, np.float32)
    b = np.asarray(b, np.float32)
    edge_index = np.asarray(edge_index, np.int32)

    if "nc" not in _compiled:
        _compiled["nc"] = _build_program()
    nc = _compiled["nc"]

    # fused score matrices: per head h, Acat[:, 64h:64h+64] = [A_h; v_h^T]
    acat = np.zeros((F + 1, 2 * F), np.float64)
    for h in range(H):
        Wq = W[:, 128 * h:128 * h + 64].astype(np.float64)
        bq = b[128 * h:128 * h + 64].astype(np.float64)
        Wk = W[:, 128 * h + 64:128 * h + 128].astype(np.float64)
        acat[:F, 64 * h:64 * h + 64] = Wq @ Wk.T
        acat[F, 64 * h:64 * h + 64] = Wk @ bq
    acat = acat.astype(np.float16)

    # ones reduction masks: lhsT for chunk c maps head h -> out row 2c+h
    ones4 = np.zeros((128, CH, 2 * CH), np.float16)
    p = np.arange(128)
    for c in range(CH):
        ones4[p, c, 2 * c + (p // 64)] = 1.0

    row = edge_index[0].astype(np.int64)
    col = edge_index[1].astype(np.int64)
    core_of = row // NLOC
    eids = np.arange(E, dtype=np.int64)
    xT = np.ascontiguousarray(x.T)  # [F, N] f32

    in_maps = []
    slot_eids = []
    for c in range(NCORES):
        msk = core_of == c
        n0 = c * NLOC
        slot_row, slot_col, slot_eid, segm = _prep_core(
            row[msk], col[msk], eids[msk], n0)
        real = slot_row >= 0
        xr_s = np.zeros((F + 1, EPAD), np.float16)
        xc_s = np.zeros((2 * F, EPAD), np.float16)
        xr_s[:F, real] = xT[:, slot_row[real]].astype(np.float16)
        xr_s[F, real] = 1.0
        xc_s[:F, real] = xT[:, slot_col[real]].astype(np.float16)
        xc_s[F:, :] = xc_s[:F, :]
        in_maps.append({
            "xr": xr_s, "xc": xc_s, "acat": acat, "ones4": ones4, "segm": segm,
        })
        slot_eids.append(slot_eid)

    res = run_bass_kernel_spmd(nc, in_maps, core_ids=list(range(NCORES)),
                               trace=bool(_compiled.get("trace")))
    _compiled["last_result"] = res

    out = np.zeros(E, np.float32)
    for c in range(NCORES):
        a = np.asarray(res.results[c]["attn"]).reshape(EPAD)
        se = slot_eids[c]
        m = se >= 0
        out[se[m]] = a[m]
    return out
